# revision 1
# baseline (speedup 1.0000x reference)
"""TRN2 Bass kernel for nn_BackMapLayer (backmapping internal coords -> cartesian).

Math: the reference sequentially rotates the atom suffix about the current
bond axis for each torsion (O(n^2), serial). Equivalent reformulation: by the
conjugation identity, the final position of atom j is C_{j-2}(p_j) where
C_k = A_0 . A_1 ... A_k is the prefix product of affine rotations about the
ORIGINAL planar-chain axes (A_i = rotation by +dih_i about axis through
planar atoms i+1 -> i+2). This is an associative scan -> Hillis-Steele
log-depth scan on chip, batch parallel across cores.

Layout per core: 32 samples x 2 sides (left-reversed / right) = 64 scan
chains on partitions; scan steps padded to 256 along the free dimension.
"""
import numpy as np
import concourse.bass as bass
import concourse.mybir as mybir
from concourse.tile import TileContext
from concourse import bass_utils

F32 = mybir.dt.float32
AL = mybir.AluOpType
AF = mybir.ActivationFunctionType
PI = float(np.pi)

B, N = 256, 512
NCORES = 8
BSH = B // NCORES            # 32 samples per core
NA, ND = N - 2, N - 3        # 510 angles, 509 dihedrals
NS = 256                     # scan steps (254/255 real + identity pads)
NAT = 258                    # side-local atoms (257 left + pad / 258 right)
ROWS = 2 * BSH               # 64 chains on partitions


def _patch_tile_drain():
    """This walrus build allows only ONE semaphore wait per instruction; the
    stock TileContext tail-drain carries one wait per logical processor.
    Split them across single-wait nops."""
    from concourse.tile import TileContext as TC
    if getattr(TC, "_drain_patched", False):
        return
    from concourse.vector_clock import ScopedClock, VectorClock
    from concourse.tile_scheduler import N_PROCS

    def _drain_and_barrier(self, tick_clock, wait_clock):
        nc = self.nc
        g = tick_clock.global_clock
        for p in range(N_PROCS):
            if g[p] > 0:
                vals = [g[q] if q == p else 0 for q in range(N_PROCS)]
                nop = nc.sync.nop(nofuse=True, hint="split_drain_wait")
                wait_clock.add_sem_waits(nop.ins, ScopedClock({None: VectorClock(vals)}))
        nc.sync.drain(fusable=False)
        nc.all_engine_barrier()
        popped = nc._tile_sem_poison_stack.pop()
        assert popped is self._sem_poison
        nc.clear_and_free_semaphores(list(self.sems.allocated().values()))
        nc.all_engine_barrier()

    TC._drain_and_barrier = _drain_and_barrier
    TC._drain_patched = True


def _patch_birsim_off():
    """The in-compile BIR simulator takes 20+ minutes on this kernel; disable."""
    if getattr(bass_utils, "_birsim_patched", False):
        return
    orig = bass_utils.run_command

    def run_command_nosim(argv, **kwargs):
        argv = [a.replace("--enable-birsim=true", "--enable-birsim=false")
                if isinstance(a, str) else a for a in argv]
        return orig(argv, **kwargs)

    bass_utils.run_command = run_command_nosim
    bass_utils._birsim_patched = True


def _patch_split_waits():
    """This walrus build rejects >1 sem wait per instruction. Rewrite the BIR
    JSON before compile: hoist extra waits onto single-wait NoOps inserted
    just before the instruction on the same engine."""
    if getattr(bass_utils, "_split_waits_patched", False):
        return
    import json as _json
    from concourse import bass2jax

    orig = bass_utils.compile_bir_kernel

    def split_waits(bir_json, *args, **kwargs):
        d = _json.loads(bir_json)
        n_split = 0
        for fn in d.get("functions", []):
            for bb in fn.get("blocks", []):
                new_insts = []
                for inst in bb.get("instructions", []):
                    si = inst.get("sync_info")
                    waits = si.get("on_wait") if si else None
                    if waits and len(waits) > 1:
                        for k, w in enumerate(waits[:-1]):
                            new_insts.append({
                                "debug": inst.get("debug", 0),
                                "engine": inst["engine"],
                                "ins": [],
                                "name": f"{inst['name']}__w{k}",
                                "opcode": "NoOp",
                                "outs": [],
                                "sync_info": {"on_update": [], "on_wait": [w]},
                            })
                        si["on_wait"] = [waits[-1]]
                        n_split += 1
                    new_insts.append(inst)
                bb["instructions"] = new_insts
        if n_split:
            bir_json = _json.dumps(d).encode()
        return orig(bir_json, *args, **kwargs)

    bass_utils.compile_bir_kernel = split_waits
    bass2jax.compile_bir_kernel = split_waits
    bass_utils._split_waits_patched = True


def _ap(tile, extra_off, dims):
    base = tile[:]
    return bass.AP(tile.tensor, base.offset + extra_off, [list(base.ap[0])] + dims)


def _rev(tile, row_hi, start, count):
    """[0:row_hi] rows, free dim read backwards from `start`, `count` elems."""
    base = tile[:]
    p = [list(base.ap[0])[0], row_hi]
    return bass.AP(tile.tensor, base.offset + start, [p, [-1, count]])


MAGIC = 12582912.0          # 1.5 * 2**23: float32 round-to-nearest-integer trick
TWO_PI = float(2 * np.pi)


def build(nc, debug=False, iters=1):
    distances = nc.dram_tensor("distances", [B, N - 1], F32, kind="ExternalInput")
    angles = nc.dram_tensor("angles", [BSH, NA], F32, kind="ExternalInput")
    dihedrals = nc.dram_tensor("dihedrals", [BSH, ND], F32, kind="ExternalInput")
    out = nc.dram_tensor("out", [BSH, N, 3], F32, kind="ExternalOutput")

    def dbg(name, tile_ap):
        if not debug:
            return
        t = nc.dram_tensor(f"dbg_{name}", list(tile_ap.shape), F32,
                           kind="ExternalOutput")
        nc.sync.dma_start(t[:], tile_ap)

    V = nc.vector

    def sin_rr(pool, out_ap, x_ap, phase, shape):
        """out = sin(x + phase), with range reduction to [-pi, pi].
        round(u) via the f32 magic-number trick (args < 2^22)."""
        a = pool.tile(shape, F32, tag="rr_a")
        if phase != 0.0:
            V.tensor_scalar_add(a[:], x_ap, phase)
            src = a[:]
        else:
            src = x_ap
        u = pool.tile(shape, F32, tag="rr_u")
        V.tensor_scalar_mul(u[:], src, 1.0 / TWO_PI)
        v = pool.tile(shape, F32, tag="rr_v")
        V.tensor_scalar_add(v[:], u[:], MAGIC)
        V.tensor_scalar_sub(v[:], v[:], MAGIC)
        r = pool.tile(shape, F32, tag="rr_r")
        V.scalar_tensor_tensor(r[:], v[:], -TWO_PI, src, AL.mult, AL.add)
        nc.scalar.activation(out_ap, r[:], AF.Sin)

    with TileContext(nc) as tc:
        with tc.tile_pool(name="sb", bufs=1) as pool, \
             tc.tile_pool(name="ps", bufs=1, space="PSUM") as psum:
          for _it in range(iters):
              # ---------- lengths = mean over full batch ----------
              dist = pool.tile([128, 2, N - 1], F32)
              nc.sync.dma_start(dist[:], distances[:].rearrange("(a p) n -> p a n", p=128))
              dsum = pool.tile([128, N - 1], F32)
              V.tensor_tensor(dsum[:], dist[:, 0, :], dist[:, 1, :], AL.add)
              ones_col = pool.tile([128, 1], F32)
              V.memset(ones_col[:], 1.0)
              len_ps = psum.tile([1, N - 1], F32)
              nc.tensor.matmul(len_ps[:], ones_col[:], dsum[:], start=True, stop=True)
              len_sb = pool.tile([1, N - 1], F32)
              nc.scalar.mul(len_sb[:], len_ps[:], 1.0 / B)
              ones_row = pool.tile([1, BSH], F32)
              V.memset(ones_row[:], 1.0)
              len_b_ps = psum.tile([BSH, N - 1], F32)
              nc.tensor.matmul(len_b_ps[:], ones_row[:], len_sb[:], start=True, stop=True)
              lengths = pool.tile([BSH, N - 1], F32)
              V.tensor_copy(lengths[:], len_b_ps[:])
              dbg('lengths', lengths[:])

              # ---------- planar chain ----------
              ang = pool.tile([BSH, NA], F32)
              nc.sync.dma_start(ang[:], angles[:])
              nang = pool.tile([BSH, NA], F32)
              V.tensor_scalar(nang[:], ang[:], -1.0, PI, AL.mult, AL.add)
              prev = pool.tile([BSH, N - 1], F32)
              V.memset(prev[:, 0:1], 0.0)
              V.tensor_tensor_scan(prev[:, 1:], nang[:], nang[:], 0.0, AL.add, AL.bypass)
              cosp = pool.tile([BSH, N - 1], F32)
              sin_rr(pool, cosp[:], prev[:], PI / 2, [BSH, N - 1])
              sinp = pool.tile([BSH, N - 1], F32)
              sin_rr(pool, sinp[:], prev[:], 0.0, [BSH, N - 1])
              dx = pool.tile([BSH, N - 1], F32)
              V.tensor_tensor(dx[:], lengths[:], cosp[:], AL.mult)
              dy = pool.tile([BSH, N - 1], F32)
              V.tensor_tensor(dy[:], lengths[:], sinp[:], AL.mult)
              xs = pool.tile([BSH, N], F32)
              ys = pool.tile([BSH, N], F32)
              V.memset(xs[:, 0:1], 0.0)
              V.memset(ys[:, 0:1], 0.0)
              V.tensor_tensor_scan(xs[:, 1:], dx[:], dx[:], 0.0, AL.add, AL.bypass)
              V.tensor_tensor_scan(ys[:, 1:], dy[:], dy[:], 0.0, AL.add, AL.bypass)
              dbg('xs', xs[:])
              dbg('ys', ys[:])

              # ---------- side-stacked planar coords ----------
              Px = pool.tile([ROWS, NAT], F32)
              Py = pool.tile([ROWS, NAT], F32)
              V.tensor_copy(Px[0:BSH, 0:257], _rev(xs, BSH, 256, 257))
              V.tensor_copy(Py[0:BSH, 0:257], _rev(ys, BSH, 256, 257))
              V.memset(Px[0:BSH, 257:258], 1.0)   # pad atom: delta (1,0) from origin
              V.memset(Py[0:BSH, 257:258], 0.0)
              V.tensor_copy(Px[BSH:ROWS, :], xs[:, 254:512])
              V.tensor_copy(Py[BSH:ROWS, :], ys[:, 254:512])
              dbg('Px', Px[:])
              dbg('Py', Py[:])

              # ---------- torsions (padded with identity steps) ----------
              dih = pool.tile([BSH, ND], F32)
              nc.sync.dma_start(dih[:], dihedrals[:])
              dihp = pool.tile([BSH, ND], F32)
              V.tensor_scalar_add(dihp[:], dih[:], PI)
              delta = pool.tile([ROWS, NS], F32)
              V.tensor_copy(delta[0:BSH, 0:254], _rev(dihp, BSH, 253, 254))
              V.memset(delta[0:BSH, 254:256], 0.0)
              V.tensor_copy(delta[BSH:ROWS, 0:255], dihp[:, 254:509])
              V.memset(delta[BSH:ROWS, 255:256], 0.0)
              dbg('delta', delta[:])

              # ---------- axes + Rodrigues build ----------
              dxv = pool.tile([ROWS, NS], F32)
              dyv = pool.tile([ROWS, NS], F32)
              V.tensor_tensor(dxv[:], Px[:, 2:258], Px[:, 1:257], AL.subtract)
              V.tensor_tensor(dyv[:], Py[:, 2:258], Py[:, 1:257], AL.subtract)
              t1 = pool.tile([ROWS, NS], F32)
              t2 = pool.tile([ROWS, NS], F32)
              t3 = pool.tile([ROWS, NS], F32)
              n2 = pool.tile([ROWS, NS], F32)
              V.tensor_tensor(t1[:], dxv[:], dxv[:], AL.mult)
              V.tensor_tensor(t2[:], dyv[:], dyv[:], AL.mult)
              V.tensor_tensor(n2[:], t1[:], t2[:], AL.add)
              nrm = pool.tile([ROWS, NS], F32)
              nc.scalar.activation(nrm[:], n2[:], AF.Sqrt)
              inv = pool.tile([ROWS, NS], F32)
              V.reciprocal(inv[:], nrm[:])
              ux = pool.tile([ROWS, NS], F32)
              uy = pool.tile([ROWS, NS], F32)
              V.tensor_tensor(ux[:], dxv[:], inv[:], AL.mult)
              V.tensor_tensor(uy[:], dyv[:], inv[:], AL.mult)
              sd = pool.tile([ROWS, NS], F32)
              cd = pool.tile([ROWS, NS], F32)
              sin_rr(pool, sd[:], delta[:], 0.0, [ROWS, NS])
              sin_rr(pool, cd[:], delta[:], PI / 2, [ROWS, NS])
              omc = pool.tile([ROWS, NS], F32)
              V.tensor_scalar(omc[:], cd[:], -1.0, 1.0, AL.mult, AL.add)
              dbg('ux', ux[:])
              dbg('uy', uy[:])
              dbg('sd', sd[:])
              dbg('cd', cd[:])

              Ra = pool.tile([ROWS, 9, NS], F32)
              uxy = pool.tile([ROWS, NS], F32)
              V.tensor_tensor(uxy[:], ux[:], uy[:], AL.mult)
              V.tensor_tensor(Ra[:, 1, :], omc[:], uxy[:], AL.mult)       # R01
              V.tensor_copy(Ra[:, 3, :], Ra[:, 1, :])                     # R10
              V.tensor_tensor(Ra[:, 2, :], sd[:], uy[:], AL.mult)         # R02
              V.scalar_tensor_tensor(Ra[:, 6, :], sd[:], -1.0, uy[:], AL.mult, AL.mult)  # R20
              V.scalar_tensor_tensor(Ra[:, 5, :], sd[:], -1.0, ux[:], AL.mult, AL.mult)  # R12
              V.tensor_tensor(Ra[:, 7, :], sd[:], ux[:], AL.mult)         # R21
              V.tensor_copy(Ra[:, 8, :], cd[:])                           # R22
              V.tensor_tensor(t1[:], ux[:], ux[:], AL.mult)
              V.tensor_tensor(t2[:], omc[:], t1[:], AL.mult)
              V.tensor_tensor(Ra[:, 0, :], t2[:], cd[:], AL.add)          # R00
              V.tensor_tensor(t1[:], uy[:], uy[:], AL.mult)
              V.tensor_tensor(t2[:], omc[:], t1[:], AL.mult)
              V.tensor_tensor(Ra[:, 4, :], t2[:], cd[:], AL.add)          # R11

              # translations t = c - R c   (c = planar atom i+2, z=0)
              Tt = pool.tile([ROWS, 3, NS], F32)
              cx = Px[:, 2:258]
              cy = Py[:, 2:258]
              V.tensor_tensor(t1[:], Ra[:, 0, :], cx, AL.mult)
              V.tensor_tensor(t2[:], Ra[:, 1, :], cy, AL.mult)
              V.tensor_tensor(t3[:], t1[:], t2[:], AL.add)
              V.tensor_tensor(Tt[:, 0, :], cx, t3[:], AL.subtract)
              V.tensor_tensor(t1[:], Ra[:, 3, :], cx, AL.mult)
              V.tensor_tensor(t2[:], Ra[:, 4, :], cy, AL.mult)
              V.tensor_tensor(t3[:], t1[:], t2[:], AL.add)
              V.tensor_tensor(Tt[:, 1, :], cy, t3[:], AL.subtract)
              V.tensor_tensor(t1[:], Ra[:, 6, :], cx, AL.mult)
              V.tensor_tensor(t2[:], Ra[:, 7, :], cy, AL.mult)
              V.scalar_tensor_tensor(Tt[:, 2, :], t1[:], -1.0, t2[:], AL.mult, AL.subtract)
              dbg('Ra', Ra[:])
              dbg('Tt', Tt[:])

              # ---------- Hillis-Steele scan of 3x3 prefix products ----------
              Rb = pool.tile([ROWS, 9, NS], F32)
              Ma = pool.tile([ROWS, 9, NS], F32)
              Mb = pool.tile([ROWS, 9, NS], F32)
              Ms = pool.tile([ROWS, 9, NS], F32)
              src, dst = Ra, Rb

              def lk(t, k, cnt):      # left operand: plane (r,k) bcast over c
                  return _ap(t, k * NS, [[3 * NS, 3], [0, 3], [1, cnt]])

              def rk(t, k, cnt, o):   # right operand: plane (k,c) bcast over r
                  return _ap(t, 3 * k * NS + o, [[0, 3], [NS, 3], [1, cnt]])

              def d9(t, cnt, o):      # all 9 planes, elems [o, o+cnt)
                  return _ap(t, o, [[3 * NS, 3], [NS, 3], [1, cnt]])

              for o in (1, 2, 4, 8, 16, 32, 64, 128):
                  cnt = NS - o
                  V.tensor_tensor(d9(Ma, cnt, o), lk(src, 0, cnt), rk(src, 0, cnt, o), AL.mult)
                  V.tensor_tensor(d9(Mb, cnt, o), lk(src, 1, cnt), rk(src, 1, cnt, o), AL.mult)
                  V.tensor_tensor(d9(Ms, cnt, o), d9(Ma, cnt, o), d9(Mb, cnt, o), AL.add)
                  V.tensor_tensor(d9(Ma, cnt, o), lk(src, 2, cnt), rk(src, 2, cnt, o), AL.mult)
                  V.tensor_tensor(d9(dst, cnt, o), d9(Ms, cnt, o), d9(Ma, cnt, o), AL.add)
                  V.tensor_copy(dst[:, :, 0:o], src[:, :, 0:o])
                  src, dst = dst, src
              Rc = src  # prefix products C_k
              dbg('Rc', Rc[:])

              # ---------- translation accumulation ----------
              # w_0 = t_0 ; w_m = C_{m-1} @ t_m ; tcum = cumsum(w)
              W = pool.tile([ROWS, 3, NS], F32)
              V.tensor_copy(W[:, :, 0:1], Tt[:, :, 0:1])
              cm = NS - 1
              for c in range(3):
                  V.tensor_tensor(t1[:, 0:cm], Rc[:, 3 * c + 0, 0:cm], Tt[:, 0, 1:NS], AL.mult)
                  V.tensor_tensor(t2[:, 0:cm], Rc[:, 3 * c + 1, 0:cm], Tt[:, 1, 1:NS], AL.mult)
                  V.tensor_tensor(t3[:, 0:cm], t1[:, 0:cm], t2[:, 0:cm], AL.add)
                  V.tensor_tensor(t1[:, 0:cm], Rc[:, 3 * c + 2, 0:cm], Tt[:, 2, 1:NS], AL.mult)
                  V.tensor_tensor(W[:, c, 1:NS], t3[:, 0:cm], t1[:, 0:cm], AL.add)
              Tc = pool.tile([ROWS, 3, NS], F32)
              for c in range(3):
                  V.tensor_tensor_scan(Tc[:, c, :], W[:, c, :], W[:, c, :], 0.0, AL.add, AL.bypass)

              dbg('W', W[:])
              dbg('Tc', Tc[:])
              # ---------- apply: pos_j = C_{j-2} p_j  (p_z = 0) ----------
              Ox = pool.tile([ROWS, NAT], F32)
              Oy = pool.tile([ROWS, NAT], F32)
              Oz = pool.tile([ROWS, NAT], F32)
              for c, Ot in ((0, Ox), (1, Oy), (2, Oz)):
                  V.tensor_tensor(t1[:], Rc[:, 3 * c + 0, :], Px[:, 2:258], AL.mult)
                  V.tensor_tensor(t2[:], Rc[:, 3 * c + 1, :], Py[:, 2:258], AL.mult)
                  V.tensor_tensor(t3[:], t1[:], t2[:], AL.add)
                  V.tensor_tensor(Ot[:, 2:258], t3[:], Tc[:, c, :], AL.add)
              V.tensor_copy(Ox[:, 0:2], Px[:, 0:2])
              V.tensor_copy(Oy[:, 0:2], Py[:, 0:2])
              V.memset(Oz[:, 0:2], 0.0)

              dbg('Ox', Ox[:])
              dbg('Oy', Oy[:])
              dbg('Oz', Oz[:])
              # ---------- output ----------
              # Interleave xyz in SBUF (left reversed to global order), then one
              # contiguous DMA. left: global atom g=256-k <- rows 0:32;
              # right: global atom 254+k <- rows 32:64, k=3..257.
              Obuf = pool.tile([BSH, N * 3], F32)
              for c, Ot in ((0, Ox), (1, Oy), (2, Oz)):
                  V.tensor_copy(_ap(Obuf, c, [[3, 257]]), _rev(Ot, BSH, 256, 257))
                  V.tensor_copy(_ap(Obuf, 257 * 3 + c, [[3, 255]]), Ot[BSH:ROWS, 3:258])
              dram_flat = bass.AP(out, 0, [[N * 3, BSH], [1, N * 3]])
              nc.sync.dma_start(dram_flat, Obuf[:])
    return nc


_prog = None


def _get_prog():
    global _prog
    if _prog is None:
        _patch_tile_drain()
        _patch_birsim_off()
        _patch_split_waits()
        nc = bass.Bass()
        _prog = build(nc)
    return _prog


TRACE = False
last_results = None


def kernel(distances, angles, dihedrals):
    global last_results
    nc = _get_prog()
    distances = np.ascontiguousarray(distances, np.float32)
    angles = np.ascontiguousarray(angles, np.float32)
    dihedrals = np.ascontiguousarray(dihedrals, np.float32)
    in_maps = []
    for c in range(NCORES):
        sl = slice(c * BSH, (c + 1) * BSH)
        in_maps.append({
            "distances": distances,
            "angles": np.ascontiguousarray(angles[sl]),
            "dihedrals": np.ascontiguousarray(dihedrals[sl]),
        })
    res = bass_utils.run_bass_kernel_spmd(
        nc, in_maps, core_ids=list(range(NCORES)), trace=TRACE
    )
    last_results = res
    return np.concatenate([res.results[c]["out"] for c in range(NCORES)], axis=0)



# revision 4
# speedup vs baseline: 2.1266x; 2.1266x over previous
"""TRN2 Bass kernel v2 for nn_BackMapLayer.

Differences vs v1 (148.6us):
- lengths = mean(distances) computed on host (the spec's "small all-reduce"),
  killing the 523KB distances DMA + on-chip mean matmuls.
- Rotation axes/centers come straight from planar cos/sin/xs/ys (the planar
  chain's segment direction IS the unit axis) — no norm/reciprocal.
- 128-partition layout: each chain split into 2 blocks of 128 scan steps
  (rows 0..63 block0 left|right, 64..127 block1), halving every free-dim.
- Brent-Kung (work-efficient) in-place prefix-product scan: 502->247 combines
  per row, 13 rounds; 3rd product of each combine runs on GPSIMD in parallel.
- Block1 results fixed up at the END by one affine transform (block0 totals).
- Elementwise work spread across Vector/GPSIMD/Scalar engines.
"""
import numpy as np
import concourse.bass as bass
import concourse.mybir as mybir
from concourse.tile import TileContext
from concourse import bass_utils

F32 = mybir.dt.float32
AL = mybir.AluOpType
AF = mybir.ActivationFunctionType
PI = float(np.pi)
MAGIC = 12582912.0          # 1.5 * 2**23 f32 round-to-int trick
TWO_PI = float(2 * np.pi)

B, N = 256, 512
NCORES = 8
BSH = B // NCORES            # 32 samples per core
NA, ND = N - 2, N - 3        # 510 angles, 509 dihedrals
NSB = 128                    # scan steps per block
ROWS = 128
PS = NSB                     # plane stride in Ra/Tt/... tiles


def _patch_tile_drain():
    from concourse.tile import TileContext as TC
    if getattr(TC, "_drain_patched", False):
        return
    from concourse.vector_clock import ScopedClock, VectorClock
    from concourse.tile_scheduler import N_PROCS

    def _drain_and_barrier(self, tick_clock, wait_clock):
        nc = self.nc
        g = tick_clock.global_clock
        for p in range(N_PROCS):
            if g[p] > 0:
                vals = [g[q] if q == p else 0 for q in range(N_PROCS)]
                nop = nc.sync.nop(nofuse=True, hint="split_drain_wait")
                wait_clock.add_sem_waits(nop.ins, ScopedClock({None: VectorClock(vals)}))
        nc.sync.drain(fusable=False)
        nc.all_engine_barrier()
        popped = nc._tile_sem_poison_stack.pop()
        assert popped is self._sem_poison
        nc.clear_and_free_semaphores(list(self.sems.allocated().values()))
        nc.all_engine_barrier()

    TC._drain_and_barrier = _drain_and_barrier
    TC._drain_patched = True


def _patch_birsim_off():
    if getattr(bass_utils, "_birsim_patched", False):
        return
    orig = bass_utils.run_command

    def run_command_nosim(argv, **kwargs):
        argv = [a.replace("--enable-birsim=true", "--enable-birsim=false")
                if isinstance(a, str) else a for a in argv]
        return orig(argv, **kwargs)

    bass_utils.run_command = run_command_nosim
    bass_utils._birsim_patched = True


def _patch_split_waits():
    if getattr(bass_utils, "_split_waits_patched", False):
        return
    import json as _json
    from concourse import bass2jax

    orig = bass_utils.compile_bir_kernel

    def split_waits(bir_json, *args, **kwargs):
        d = _json.loads(bir_json)
        n_split = 0
        for fn in d.get("functions", []):
            for bb in fn.get("blocks", []):
                new_insts = []
                for inst in bb.get("instructions", []):
                    si = inst.get("sync_info")
                    waits = si.get("on_wait") if si else None
                    if waits and len(waits) > 1:
                        for k, w in enumerate(waits[:-1]):
                            new_insts.append({
                                "debug": inst.get("debug", 0),
                                "engine": inst["engine"],
                                "ins": [],
                                "name": f"{inst['name']}__w{k}",
                                "opcode": "NoOp",
                                "outs": [],
                                "sync_info": {"on_update": [], "on_wait": [w]},
                            })
                        si["on_wait"] = [waits[-1]]
                        n_split += 1
                    new_insts.append(inst)
                bb["instructions"] = new_insts
        if n_split:
            bir_json = _json.dumps(d).encode()
        return orig(bir_json, *args, **kwargs)

    bass_utils.compile_bir_kernel = split_waits
    bass2jax.compile_bir_kernel = split_waits
    bass_utils._split_waits_patched = True


def rows(tile, rs, rn, off, dims):
    """AP over rows [rs, rs+rn) with free-dim pattern `dims` at elem `off`."""
    base = tile[:]
    ps = list(base.ap[0])[0]
    return bass.AP(tile.tensor, base.offset + rs * ps + off,
                   [[ps, rn]] + [list(d) for d in dims])


CLF = float(np.nextafter(np.float32(np.pi), np.float32(0)))  # largest f32 < pi
PAD = -CLF
HPI = float(np.float32(np.pi / 2))


def _register_consts(nc, vals):
    for v in vals:
        key = (F32, float(v))
        if key in nc.const_aps.aps:
            continue
        t = nc.alloc_sbuf_tensor(f"const-f32-{float(v)}", [128, 1], F32)
        nc.gpsimd.memset(t.ap(), float(v))
        nc.const_aps.aps[key] = t.ap()
    nc.all_engine_barrier()


def build(nc, debug=False, stop=99):
    _register_consts(nc, [HPI, -HPI])
    angles = nc.dram_tensor("angles", [BSH, NA], F32, kind="ExternalInput")
    dihedrals = nc.dram_tensor("dihedrals", [BSH, ND], F32, kind="ExternalInput")
    lengths = nc.dram_tensor("lengths", [BSH, N - 1], F32, kind="ExternalInput")
    out = nc.dram_tensor("out", [BSH, N, 3], F32, kind="ExternalOutput")

    V, G, S = nc.vector, nc.gpsimd, nc.scalar

    def dbg(name, tile_ap):
        if not debug:
            return
        t = nc.dram_tensor(f"dbg_{name}", list(tile_ap.shape), F32,
                           kind="ExternalOutput")
        nc.sync.dma_start(t[:], tile_ap)

    with TileContext(nc) as tc:
        with tc.tile_pool(name="sb", bufs=1) as pool:
            def _early_out():
                dummy = pool.tile([BSH, N * 3], F32)
                V.memset(dummy[:], 0.0)
                df = bass.AP(out, 0, [[N * 3, BSH], [1, N * 3]])
                nc.sync.dma_start(df, dummy[:])

            # ---------------- input DMAs ----------------
            ang = pool.tile([BSH, NA], F32)
            dih = pool.tile([BSH, ND], F32)
            lenr = pool.tile([BSH, N - 1], F32)
            nc.sync.dma_start(ang[:], angles[:])
            nc.sync.dma_start(dih[:], dihedrals[:])
            nc.sync.dma_start(lenr[:], lengths[:])

            # ---------------- planar chain (rows 0..31) ----------------
            nang = pool.tile([BSH, NA], F32)
            V.tensor_scalar(nang[:], ang[:], -1.0, PI, AL.mult, AL.add)
            pq = pool.tile([BSH, N - 1], F32)     # prev
            V.memset(pq[:, 0:1], 0.0)
            V.tensor_tensor_scan(pq[:, 1:], nang[:], nang[:], 0.0,
                                 AL.add, AL.bypass)
            v = pool.tile([BSH, N - 1], F32)
            V.tensor_scalar(v[:], pq[:], 1.0 / TWO_PI, MAGIC, AL.mult, AL.add)
            V.tensor_scalar_sub(v[:], v[:], MAGIC)
            rr = pool.tile([BSH, N - 1], F32)     # prev reduced to [-pi,pi]
            V.scalar_tensor_tensor(rr[:], v[:], -TWO_PI, pq[:], AL.mult, AL.add)
            V.tensor_scalar_min(rr[:], rr[:], CLF)
            V.tensor_scalar_max(rr[:], rr[:], -CLF)
            sinp = pool.tile([BSH, N - 1], F32)
            cosp = pool.tile([BSH, N - 1], F32)
            rabs = pool.tile([BSH, N - 1], F32)
            S.activation(sinp[:], rr[:], AF.Sin)
            S.activation(rabs[:], rr[:], AF.Abs)
            S.activation(cosp[:], rabs[:], AF.Sin, bias=HPI, scale=-1.0)
            dxl = pool.tile([BSH, N - 1], F32)
            dyl = pool.tile([BSH, N - 1], F32)
            V.tensor_tensor(dxl[:], lenr[:], cosp[:], AL.mult)
            V.tensor_tensor(dyl[:], lenr[:], sinp[:], AL.mult)
            xs = pool.tile([BSH, N], F32)
            ys = pool.tile([BSH, N], F32)
            V.memset(xs[:, 0:1], 0.0)
            V.memset(ys[:, 0:1], 0.0)
            V.tensor_tensor_scan(xs[:, 1:], dxl[:], dxl[:], 0.0, AL.add, AL.bypass)
            V.tensor_tensor_scan(ys[:, 1:], dyl[:], dyl[:], 0.0, AL.add, AL.bypass)
            dbg('xs', xs[:]); dbg('ys', ys[:])

            if stop <= 1:
                _early_out()
                return nc

            # ---------------- torsion raw values (GPSIMD copies) ----------
            raw = pool.tile([ROWS, NSB], F32)
            G.memset(rows(raw, 64, 32, 126, [[1, 2]]), PAD)
            G.memset(rows(raw, 96, 32, 127, [[1, 1]]), PAD)
            G.tensor_copy(rows(raw, 0, 32, 0, [[1, 128]]),
                          rows(dih, 0, 32, 253, [[-1, 128]]))
            G.tensor_copy(rows(raw, 32, 32, 0, [[1, 128]]),
                          rows(dih, 0, 32, 254, [[1, 128]]))
            G.tensor_copy(rows(raw, 64, 32, 0, [[1, 126]]),
                          rows(dih, 0, 32, 125, [[-1, 126]]))
            G.tensor_copy(rows(raw, 96, 32, 0, [[1, 127]]),
                          rows(dih, 0, 32, 382, [[1, 127]]))
            dbg('raw', raw[:])

            # sd/cd via ScalarE Sin (left: sin(raw), right: sin(-raw);
            # cd = sin(-raw - pi/2) for both)
            sd = pool.tile([ROWS, NSB], F32)
            cd = pool.tile([ROWS, NSB], F32)
            S.activation(sd[0:32], raw[0:32], AF.Sin)
            S.activation(sd[32:64], raw[32:64], AF.Sin, scale=-1.0)
            S.activation(sd[64:96], raw[64:96], AF.Sin)
            S.activation(sd[96:128], raw[96:128], AF.Sin, scale=-1.0)
            S.activation(cd[:], raw[:], AF.Sin, bias=-HPI, scale=-1.0)
            dbg('sd', sd[:]); dbg('cd', cd[:])

            if stop <= 2:
                _early_out()
                return nc

            # ---------------- axis/center stacking ----------------
            UX = pool.tile([ROWS, NSB], F32)
            UY = pool.tile([ROWS, NSB], F32)
            CX = pool.tile([ROWS, NSB], F32)
            CY = pool.tile([ROWS, NSB], F32)
            G.memset(rows(UX, 64, 32, 127, [[1, 1]]), 0.0)
            G.memset(rows(UY, 64, 32, 127, [[1, 1]]), 0.0)
            G.memset(rows(CX, 64, 32, 127, [[1, 1]]), 0.0)
            G.memset(rows(CY, 64, 32, 127, [[1, 1]]), 0.0)
            S.copy(rows(UX, 0, 32, 0, [[1, 128]]), rows(cosp, 0, 32, 254, [[-1, 128]]))
            S.copy(rows(UY, 0, 32, 0, [[1, 128]]), rows(sinp, 0, 32, 254, [[-1, 128]]))
            G.tensor_copy(rows(UX, 32, 32, 0, [[1, 128]]), rows(cosp, 0, 32, 255, [[1, 128]]))
            G.tensor_copy(rows(UY, 32, 32, 0, [[1, 128]]), rows(sinp, 0, 32, 255, [[1, 128]]))
            S.copy(rows(UX, 64, 32, 0, [[1, 127]]), rows(cosp, 0, 32, 126, [[-1, 127]]))
            S.copy(rows(UY, 64, 32, 0, [[1, 127]]), rows(sinp, 0, 32, 126, [[-1, 127]]))
            G.tensor_copy(rows(UX, 96, 32, 0, [[1, 128]]), rows(cosp, 0, 32, 383, [[1, 128]]))
            G.tensor_copy(rows(UY, 96, 32, 0, [[1, 128]]), rows(sinp, 0, 32, 383, [[1, 128]]))
            S.copy(rows(CX, 0, 32, 0, [[1, 128]]), rows(xs, 0, 32, 254, [[-1, 128]]))
            S.copy(rows(CY, 0, 32, 0, [[1, 128]]), rows(ys, 0, 32, 254, [[-1, 128]]))
            G.tensor_copy(rows(CX, 32, 32, 0, [[1, 128]]), rows(xs, 0, 32, 256, [[1, 128]]))
            G.tensor_copy(rows(CY, 32, 32, 0, [[1, 128]]), rows(ys, 0, 32, 256, [[1, 128]]))
            S.copy(rows(CX, 64, 32, 0, [[1, 127]]), rows(xs, 0, 32, 126, [[-1, 127]]))
            S.copy(rows(CY, 64, 32, 0, [[1, 127]]), rows(ys, 0, 32, 126, [[-1, 127]]))
            G.tensor_copy(rows(CX, 96, 32, 0, [[1, 128]]), rows(xs, 0, 32, 384, [[1, 128]]))
            G.tensor_copy(rows(CY, 96, 32, 0, [[1, 128]]), rows(ys, 0, 32, 384, [[1, 128]]))
            dbg('UX', UX[:]); dbg('UY', UY[:]); dbg('CX', CX[:]); dbg('CY', CY[:])

            if stop <= 3:
                _early_out()
                return nc

            # ---------------- Rodrigues build ----------------
            omc = pool.tile([ROWS, NSB], F32)
            x2 = pool.tile([ROWS, NSB], F32)
            uxy = pool.tile([ROWS, NSB], F32)
            tt0 = pool.tile([ROWS, NSB], F32)
            G.tensor_scalar(omc[:], cd[:], -1.0, 1.0, AL.mult, AL.add)
            S.square(x2[:], UX[:])
            V.tensor_tensor(uxy[:], UX[:], UY[:], AL.mult)
            Ra = pool.tile([ROWS, 9, NSB], F32)
            V.tensor_tensor(tt0[:], omc[:], x2[:], AL.mult)
            V.tensor_tensor(Ra[:, 0, :], tt0[:], cd[:], AL.add)      # R00
            V.tensor_tensor(Ra[:, 1, :], omc[:], uxy[:], AL.mult)    # R01
            V.tensor_tensor(Ra[:, 2, :], sd[:], UY[:], AL.mult)      # R02
            S.copy(Ra[:, 3, :], Ra[:, 1, :])                         # R10
            G.tensor_scalar(Ra[:, 4, :], tt0[:], -1.0, 1.0, AL.mult, AL.add)  # R11
            V.tensor_tensor(Ra[:, 7, :], sd[:], UX[:], AL.mult)      # R21
            S.mul(Ra[:, 5, :], Ra[:, 7, :], -1.0)                    # R12
            S.mul(Ra[:, 6, :], Ra[:, 2, :], -1.0)                    # R20
            S.copy(Ra[:, 8, :], cd[:])                               # R22
            dbg('Ra', Ra[:])

            if stop <= 4:
                _early_out()
                return nc

            # ---------------- translations t = c - R c ----------------
            Tt = pool.tile([ROWS, 3, NSB], F32)
            ta = pool.tile([ROWS, NSB], F32)
            tb = pool.tile([ROWS, NSB], F32)
            tb2 = pool.tile([ROWS, NSB], F32)
            tg1 = pool.tile([ROWS, NSB], F32)
            tg2 = pool.tile([ROWS, NSB], F32)
            V.tensor_tensor(ta[:], Ra[:, 0, :], CX[:], AL.mult)
            G.tensor_tensor(tb[:], Ra[:, 1, :], CY[:], AL.mult)
            V.tensor_tensor(ta[:], ta[:], tb[:], AL.add)
            V.tensor_tensor(Tt[:, 0, :], CX[:], ta[:], AL.subtract)
            V.tensor_tensor(ta[:], Ra[:, 1, :], CX[:], AL.mult)
            G.tensor_tensor(tb2[:], Ra[:, 4, :], CY[:], AL.mult)
            V.tensor_tensor(ta[:], ta[:], tb2[:], AL.add)
            V.tensor_tensor(Tt[:, 1, :], CY[:], ta[:], AL.subtract)
            G.tensor_tensor(tg1[:], Ra[:, 2, :], CX[:], AL.mult)
            G.tensor_tensor(tg2[:], Ra[:, 7, :], CY[:], AL.mult)
            V.tensor_tensor(Tt[:, 2, :], tg1[:], tg2[:], AL.subtract)
            dbg('Tt', Tt[:])

            if stop <= 5:
                _early_out()
                return nc

            # ---------------- Brent-Kung prefix-product scan (in place) ---
            def lk(t, k, eoff, stride, cnt):    # entry (r,k), bcast over c
                base = t[:]
                ps = list(base.ap[0])[0]
                return bass.AP(t.tensor, base.offset + k * PS + eoff,
                               [[ps, ROWS], [3 * PS, 3], [0, 3], [stride, cnt]])

            def rk(t, k, eoff, stride, cnt):    # entry (k,c), bcast over r
                base = t[:]
                ps = list(base.ap[0])[0]
                return bass.AP(t.tensor, base.offset + 3 * k * PS + eoff,
                               [[ps, ROWS], [0, 3], [PS, 3], [stride, cnt]])

            def d9(t, eoff, stride, cnt):       # all 9 entries strided
                base = t[:]
                ps = list(base.ap[0])[0]
                return bass.AP(t.tensor, base.offset + eoff,
                               [[ps, ROWS], [3 * PS, 3], [PS, 3], [stride, cnt]])

            def d9c(t, cnt):                    # compact temp [ROWS,9,64]
                base = t[:]
                ps = list(base.ap[0])[0]
                return bass.AP(t.tensor, base.offset,
                               [[ps, ROWS], [192, 3], [64, 3], [1, cnt]])

            Ma = pool.tile([ROWS, 9, 64], F32)
            Mb = pool.tile([ROWS, 9, 64], F32)
            Mg = pool.tile([ROWS, 9, 64], F32)
            rounds = [("u", d) for d in (1, 2, 4, 8, 16, 32, 64)] + \
                     [("d", d) for d in (32, 16, 8, 4, 2, 1)]
            for phase, d in rounds:
                if phase == "u":
                    e_dst = 2 * d - 1
                    cnt = NSB // (2 * d)
                else:
                    e_dst = 3 * d - 1
                    cnt = NSB // (2 * d) - 1
                e_l = e_dst - d
                st = 2 * d
                V.tensor_tensor(d9c(Ma, cnt), lk(Ra, 0, e_l, st, cnt),
                                rk(Ra, 0, e_dst, st, cnt), AL.mult)
                V.tensor_tensor(d9c(Mb, cnt), lk(Ra, 1, e_l, st, cnt),
                                rk(Ra, 1, e_dst, st, cnt), AL.mult)
                V.tensor_tensor(d9c(Ma, cnt), d9c(Ma, cnt), d9c(Mb, cnt), AL.add)
                G.tensor_tensor(d9c(Mg, cnt), lk(Ra, 2, e_l, st, cnt),
                                rk(Ra, 2, e_dst, st, cnt), AL.mult)
                V.tensor_tensor(d9(Ra, e_dst, st, cnt), d9c(Ma, cnt),
                                d9c(Mg, cnt), AL.add)
            dbg('Rc', Ra[:])

            if stop <= 6:
                _early_out()
                return nc

            # ---------------- translation accumulation ----------------
            # W[m] = C[m-1] @ t[m] (m>=1), W[0]=t[0]; Tc = cumsum(W)
            W = pool.tile([ROWS, 3, NSB], F32)
            Wa = pool.tile([ROWS, 3, 127], F32)
            Wg = pool.tile([ROWS, 3, 127], F32)
            S.copy(W[:, :, 0:1], Tt[:, :, 0:1])

            def lkW(k):     # C entry (c,k) over c, elems 0..126
                base = Ra[:]
                ps = list(base.ap[0])[0]
                return bass.AP(Ra.tensor, base.offset + k * PS,
                               [[ps, ROWS], [3 * PS, 3], [1, 127]])

            def rkW(k):     # Tt plane k bcast over c, elems 1..127
                base = Tt[:]
                ps = list(base.ap[0])[0]
                return bass.AP(Tt.tensor, base.offset + k * PS + 1,
                               [[ps, ROWS], [0, 3], [1, 127]])

            V.tensor_tensor(Wa[:], lkW(0), rkW(0), AL.mult)
            G.tensor_tensor(Wg[:], lkW(2), rkW(2), AL.mult)
            Wb = pool.tile([ROWS, 3, 127], F32)
            V.tensor_tensor(Wb[:], lkW(1), rkW(1), AL.mult)
            V.tensor_tensor(Wa[:], Wa[:], Wb[:], AL.add)
            V.tensor_tensor(W[:, :, 1:], Wa[:], Wg[:], AL.add)
            Tc = pool.tile([ROWS, 3, NSB], F32)
            V.tensor_tensor_scan(Tc[:, 0, :], W[:, 0, :], W[:, 0, :], 0.0,
                                 AL.add, AL.bypass)
            V.tensor_tensor_scan(Tc[:, 1, :], W[:, 1, :], W[:, 1, :], 0.0,
                                 AL.add, AL.bypass)
            V.tensor_tensor_scan(Tc[:, 2, :], W[:, 2, :], W[:, 2, :], 0.0,
                                 AL.add, AL.bypass)  # GPS scan breaks walrus?
            dbg('Tc', Tc[:])

            if stop <= 7:
                _early_out()
                return nc

            # ---------------- apply: O[c] = C[c,0] CX + C[c,1] CY + Tc[c] --
            O = pool.tile([ROWS, 3, NSB], F32)
            Oa = pool.tile([ROWS, 3, NSB], F32)
            Ob = pool.tile([ROWS, 3, NSB], F32)

            def lkA(k, rs=0, rn=ROWS):
                base = Ra[:]
                ps = list(base.ap[0])[0]
                return bass.AP(Ra.tensor, base.offset + rs * ps + k * PS,
                               [[ps, rn], [3 * PS, 3], [1, NSB]])

            def bc3(t, rs=0, rn=ROWS, eoff=0):  # [ROWS,NSB] tile bcast over c
                base = t[:]
                ps = list(base.ap[0])[0]
                return bass.AP(t.tensor, base.offset + rs * ps + eoff,
                               [[ps, rn], [0, 3], [1, NSB]])

            V.tensor_tensor(Oa[:], lkA(0), bc3(CX), AL.mult)
            G.tensor_tensor(Ob[:], lkA(1), bc3(CY), AL.mult)
            V.tensor_tensor(Oa[:], Oa[:], Ob[:], AL.add)
            V.tensor_tensor(O[:], Oa[:], Tc[:], AL.add)

            if stop <= 8:
                _early_out()
                return nc

            # ---------------- block1 affine fixup ----------------
            R9 = pool.tile([ROWS, 9], F32)
            T3 = pool.tile([ROWS, 3], F32)
            G.tensor_copy(rows(R9, 64, 64, 0, [[1, 9]]),
                          rows(Ra, 0, 64, NSB - 1, [[PS, 9]]))
            G.tensor_copy(rows(T3, 64, 64, 0, [[1, 3]]),
                          rows(Tc, 0, 64, NSB - 1, [[PS, 3]]))
            Fa = pool.tile([ROWS, 3, NSB], F32)
            Fb = pool.tile([ROWS, 3, NSB], F32)
            Fg = pool.tile([ROWS, 3, NSB], F32)

            def lkF(k):     # R9 entry (c,k) bcast over elems, rows 64..127
                base = R9[:]
                ps = list(base.ap[0])[0]
                return bass.AP(R9.tensor, base.offset + 64 * ps + k,
                               [[ps, 64], [3, 3], [0, NSB]])

            def rkO(k):     # O plane k bcast over c, rows 64..127
                base = O[:]
                ps = list(base.ap[0])[0]
                return bass.AP(O.tensor, base.offset + 64 * ps + k * PS,
                               [[ps, 64], [0, 3], [1, NSB]])

            V.tensor_tensor(Fa[64:128], lkF(0), rkO(0), AL.mult)
            G.tensor_tensor(Fb[64:128], lkF(1), rkO(1), AL.mult)
            V.tensor_tensor(Fa[64:128], Fa[64:128], Fb[64:128], AL.add)
            G.tensor_tensor(Fg[64:128], lkF(2), rkO(2), AL.mult)
            V.tensor_tensor(Fa[64:128], Fa[64:128], Fg[64:128], AL.add)

            def bcT3():     # T3 (c) bcast over elems, rows 64..127
                base = T3[:]
                ps = list(base.ap[0])[0]
                return bass.AP(T3.tensor, base.offset + 64 * ps,
                               [[ps, 64], [1, 3], [0, NSB]])

            V.tensor_tensor(O[64:128], Fa[64:128], bcT3(), AL.add)
            dbg('O', O[:])

            if stop <= 9:
                _early_out()
                return nc

            # ---------------- output interleave + DMA ----------------
            Obuf = pool.tile([BSH, N * 3], F32)
            for c, eng in ((0, V), (1, S), (2, G)):
                copy = eng.tensor_copy if eng is not S else eng.copy
                # leftB0: f -> atom 254-f
                copy(rows(Obuf, 0, 32, 3 * 254 + c, [[-3, 128]]),
                     rows(O, 0, 32, c * PS, [[1, 128]]))
                # leftB1: f=0..126 -> atom 126-f
                copy(rows(Obuf, 0, 32, 3 * 126 + c, [[-3, 127]]),
                     rows(O, 64, 32, c * PS, [[1, 127]]))
                # rightB0: f=1..127 -> atom 257..383
                copy(rows(Obuf, 0, 32, 3 * 257 + c, [[3, 127]]),
                     rows(O, 32, 32, c * PS + 1, [[1, 127]]))
                # rightB1: f=0..127 -> atom 384..511
                copy(rows(Obuf, 0, 32, 3 * 384 + c, [[3, 128]]),
                     rows(O, 96, 32, c * PS, [[1, 128]]))
            # atoms 255,256 directly from planar
            V.tensor_copy(rows(Obuf, 0, 32, 3 * 255 + 0, [[3, 2]]),
                          rows(xs, 0, 32, 255, [[1, 2]]))
            V.tensor_copy(rows(Obuf, 0, 32, 3 * 255 + 1, [[3, 2]]),
                          rows(ys, 0, 32, 255, [[1, 2]]))
            V.memset(rows(Obuf, 0, 32, 3 * 255 + 2, [[3, 2]]), 0.0)

            dram_flat = bass.AP(out, 0, [[N * 3, BSH], [1, N * 3]])
            nc.sync.dma_start(dram_flat, Obuf[:])
    return nc


_prog = None


def _get_prog():
    global _prog
    if _prog is None:
        _patch_tile_drain()
        _patch_birsim_off()
        _patch_split_waits()
        nc = bass.Bass()
        _prog = build(nc)
    return _prog


TRACE = False
last_results = None


def kernel(distances, angles, dihedrals):
    global last_results
    nc = _get_prog()
    distances = np.ascontiguousarray(distances, np.float32)
    angles = np.ascontiguousarray(angles, np.float32)
    dihedrals = np.ascontiguousarray(dihedrals, np.float32)
    lengths_row = distances.mean(0, dtype=np.float32)            # [N-1]
    lengths_rep = np.ascontiguousarray(
        np.broadcast_to(lengths_row, (BSH, N - 1)), np.float32)
    in_maps = []
    for c in range(NCORES):
        sl = slice(c * BSH, (c + 1) * BSH)
        in_maps.append({
            "angles": np.ascontiguousarray(angles[sl]),
            "dihedrals": np.ascontiguousarray(dihedrals[sl]),
            "lengths": lengths_rep,
        })
    res = bass_utils.run_bass_kernel_spmd(
        nc, in_maps, core_ids=list(range(NCORES)), trace=TRACE
    )
    last_results = res
    return np.concatenate([res.results[c]["out"] for c in range(NCORES)], axis=0)
